# revision 1
# baseline (speedup 1.0000x reference)
"""Trainium2 Bass kernel for the CurvatureConstraint (marching-cubes curvature
loss) problem. Self-contained: rebuilds the deterministic topology tables,
compiles an 8-core SPMD Bass/Tile kernel, shards cells over the W axis, and
host-reduces the per-core partial accumulators to the scalar loss.

Math (validated vs reference to ~2e-4 in fp32):
  Per cell, triangle (cfg,t) with edges (e0,e1,e2):
    d1 = v(e1)-v(e0), d2 = v(e2)-v(e0)      (linear in the 12 edge offsets)
    n = d1 x d2
    ns2_t = |n|^2 = <d1,d1><d2,d2> - <d1,d2>^2            (Lagrange identity)
    <n_t,n_u> = <d1t,d1u><d2t,d2u> - <d1t,d2u><d2t,d1u>
    cos_p = <n_t,n_u> / sqrt(ns2_t*ns2_u)
    curv[cfg] = npairs[cfg] - sum_p pm_p cos_p
    loss = sum_{cell,cfg} topo[cell, g_cfg] * curv[cell,cfg]

  Every q = <dA,dB> is quadratic in the offsets -> one PE matmul
  F[cells,91] @ Mmat[91, 96*28] produces, per cfg, srcL|srcR blocks whose
  elementwise product P gives [q11q22(4), AD(3), BC(3), q12^2(4)].
  (A,B columns are pre-scaled by the pair mask pm.)
  DVE: P, ns2 = P0-P10, num = P4-P7, den = shifted-window product (clamped),
  rr = sqrt(recip_approx(den)) via DVE+ACT, cosw = num*rr, z = group-sum.
  PE: acc[97,256] += [z|1]^T @ topo_tile accumulated over all cell tiles.
  Host: loss = sum(w1 * acc[96]) - sum_cfg acc[cfg, g_cfg], summed over cores.
"""
import os
import sys
import numpy as np

for _p in ("/opt/trn_rl_repo",):
    if _p not in sys.path and os.path.isdir(_p):
        sys.path.append(_p)

# ----------------------------------------------------------------------------
# Problem constants and deterministic tables (match reference.py exactly)
# ----------------------------------------------------------------------------
W = H = D = 40
T = 256
NCFG = 96
MAXT = 4
N = W * H * D

_rs = np.random.RandomState(0)
TOPO2TRI = _rs.randint(0, T, size=NCFG)
TRI_EDGES = _rs.rand(NCFG, MAXT, 12).argsort(-1)[..., :3]
_NTRI = _rs.randint(1, MAXT + 1, size=NCFG)
TRI_MASK = (np.arange(MAXT)[None, :] < _NTRI[:, None]).astype(np.float32)
PAIR_MASK = TRI_MASK[:, :-1] * TRI_MASK[:, 1:]

EDGES = [(0,0,0,0),(0,1,0,0),(0,0,1,0),(0,1,1,0),
         (0,0,0,1),(1,0,0,1),(0,0,1,1),(1,0,1,1),
         (0,0,0,2),(1,0,0,2),(0,1,0,2),(1,1,0,2)]
CORNER = np.array([[dx, dy, dz] for dx, dy, dz, ax in EDGES], dtype=np.float64)
AXIS_OF = np.array([ax for dx, dy, dz, ax in EDGES], dtype=np.int64)
AXES = np.eye(3)

NCORES = 8
WS = W // NCORES            # 5 planes of cells per core
CELLS = WS * H * D          # 8000
QW = 28
QWID = NCFG * QW            # 2688
DEN_CLAMP = 1e-7

# ---------------- feature basis: [1, o_e(12), o_a*o_b (PAIRS)] ----------------
def _build_pairs():
    need = set()

    def add(eA, eB):
        for x in eA:
            for y in eB:
                need.add((min(x, y), max(x, y)))

    for cfg in range(NCFG):
        tri = TRI_EDGES[cfg]
        for t in range(MAXT):
            e0, e1, e2 = tri[t]
            add((e0, e1), (e0, e1))
            add((e0, e2), (e0, e2))
            add((e0, e1), (e0, e2))
        for p in range(MAXT - 1):
            e0t, e1t, e2t = tri[p]
            e0u, e1u, e2u = tri[p + 1]
            add((e0t, e1t), (e0u, e1u))
            add((e0t, e2t), (e0u, e2u))
            add((e0t, e1t), (e0u, e2u))
            add((e0t, e2t), (e0u, e1u))
    return sorted(need)

PAIRS = _build_pairs()
NPAIRF = len(PAIRS)         # 78
NF = 13 + NPAIRF            # 91
PAIR_IDX = {p: 13 + i for i, p in enumerate(PAIRS)}


def _lin_form(e0, e1):
    c = CORNER[e1] - CORNER[e0]
    coeffs = {}
    coeffs[e1] = coeffs.get(e1, np.zeros(3)) + AXES[AXIS_OF[e1]]
    coeffs[e0] = coeffs.get(e0, np.zeros(3)) - AXES[AXIS_OF[e0]]
    return c, coeffs


def _dot_poly(fA, fB):
    cA, mA = fA
    cB, mB = fB
    v = np.zeros(NF)
    v[0] = cA @ cB
    for e, ca in mA.items():
        v[1 + e] += ca @ cB
    for e, cb in mB.items():
        v[1 + e] += cA @ cb
    for ea, ca in mA.items():
        for eb, cb in mB.items():
            v[PAIR_IDX[(min(ea, eb), max(ea, eb))]] += ca @ cb
    return v


def _build_mmat():
    M = np.zeros((NF, QWID))
    for cfg in range(NCFG):
        base = cfg * QW
        d1 = [_lin_form(*TRI_EDGES[cfg, t][[0, 1]]) for t in range(MAXT)]
        d2 = [_lin_form(*TRI_EDGES[cfg, t][[0, 2]]) for t in range(MAXT)]
        for t in range(MAXT):
            M[:, base + 0 + t] = _dot_poly(d1[t], d1[t])      # q11 -> srcL[0:4]
            M[:, base + 14 + t] = _dot_poly(d2[t], d2[t])     # q22 -> srcR[0:4]
            q12 = _dot_poly(d1[t], d2[t])
            M[:, base + 10 + t] = q12                          # srcL[10:14]
            M[:, base + 24 + t] = q12                          # srcR[10:14]
        for p in range(MAXT - 1):
            t, u = p, p + 1
            pm = float(PAIR_MASK[cfg, p])
            M[:, base + 4 + p] = pm * _dot_poly(d1[t], d1[u])   # A
            M[:, base + 18 + p] = _dot_poly(d2[t], d2[u])       # Dd
            M[:, base + 7 + p] = pm * _dot_poly(d1[t], d2[u])   # B
            M[:, base + 21 + p] = _dot_poly(d2[t], d1[u])       # C
    return M.astype(np.float32)

MMAT_BASE = _build_mmat()

# Device layout: non-DMA SBUF access patterns must start at partition
# 0/32/64/96 with row-count limits (78 rows -> must start at 0), and DVE
# lanes are partition-local. So products live at rows 0..77, the ones row
# (DMA-fed) at 78, and the 12 offset rows at 79..90.
NFD = NF                      # 91
MMAT_DEV = np.zeros((NFD, QWID), dtype=np.float32)
MMAT_DEV[0:NPAIRF] = MMAT_BASE[13:13 + NPAIRF]
MMAT_DEV[NPAIRF] = MMAT_BASE[0]
MMAT_DEV[NPAIRF + 1:NPAIRF + 13] = MMAT_BASE[1:13]

NPAIRS_CFG = PAIR_MASK.sum(-1)                    # [96]
W1 = np.zeros(T, dtype=np.float64)
for _cfg in range(NCFG):
    W1[TOPO2TRI[_cfg]] += NPAIRS_CFG[_cfg]

# ----------------------------------------------------------------------------
# Bass kernel
# ----------------------------------------------------------------------------
_CACHE = {}


def _build_bass(nplanes=WS, dbg=False):
    import concourse.bass as bass
    import concourse.tile as tile
    import bass_rust
    from concourse import mybir
    from contextlib import ExitStack

    f32 = mybir.dt.float32
    cells = nplanes * H * D
    ntiles = (cells + 127) // 128
    sizes = [128] * (cells // 128) + ([cells % 128] if cells % 128 else [])

    nc = bass.Bass()
    lab_d = nc.dram_tensor("lab", [NFD, 2 * cells], f32,
                           kind="ExternalInput")
    topo_d = nc.dram_tensor("topo", [cells, T], f32, kind="ExternalInput")
    mmat_d = nc.dram_tensor("mmat", [NFD, QWID], f32, kind="ExternalInput")
    out_d = nc.dram_tensor("out", [NCFG + 1, T], f32, kind="ExternalOutput")
    dbg_d = None
    if dbg:
        dbg_d = nc.dram_tensor("dbgcos", [128, NCFG * 4], f32,
                               kind="ExternalOutput")
        dbgq_d = nc.dram_tensor("dbgq", [128, QWID], f32,
                                kind="ExternalOutput")

    with ExitStack() as ctx:
        tc = ctx.enter_context(tile.TileContext(nc))
        const = ctx.enter_context(tc.tile_pool(name="const", bufs=1))
        work = ctx.enter_context(tc.tile_pool(name="work", bufs=1))
        topop = ctx.enter_context(tc.tile_pool(name="topop", bufs=3))
        srp = ctx.enter_context(tc.tile_pool(name="srp", bufs=3))
        qpool = ctx.enter_context(tc.tile_pool(name="qp", bufs=2, space="PSUM"))
        accp = ctx.enter_context(tc.tile_pool(name="accp", bufs=1, space="PSUM"))

        mmat = const.tile([NFD, QWID], f32)
        nc.sync.dma_start(mmat[:], mmat_d[:])

        ft = const.tile([NFD, cells], f32)
        lab = const.tile([NFD, 2 * cells], f32)

        nc.sync.dma_start(lab[:], lab_d[:])
        # Codegen allows at most ONE semaphore wait per instruction, so
        # cross-engine deps must be covered transitively through engine
        # clocks. A 1-element "touch" makes DVE observe the mmat DMA before
        # the ft product; the first matmul then needs only the DVE wait.
        scratch = work.tile([1, 4], f32)
        touch = nc.vector.tensor_copy(scratch[0:1, 0:1], mmat[0:1, 0:1])
        prod = nc.vector.tensor_mul(ft[:, :], lab[:, 0:cells],
                                    lab[:, cells:2 * cells])
        bass_rust.add_dep_helper(prod.ins, touch.ins, sync=False,
                                 reason="order mmat-touch before ft product")

        # persistent per-tile intermediates (DVE-serial, single buffers)
        pt = work.tile([128, NCFG, 14], f32)
        ns2 = work.tile([128, NCFG * 4 + 1], f32)
        num = work.tile([128, NCFG, 4], f32)
        den = work.tile([128, NCFG * 4], f32)
        lnd = work.tile([128, NCFG * 4], f32)
        rr = work.tile([128, NCFG * 4], f32)
        cosw = work.tile([128, NCFG, 4], f32)
        zt = work.tile([128, NCFG + 1], f32)
        nc.vector.memset(num[:], 0.0)
        nc.vector.memset(ns2[:, NCFG * 4:], 1.0)
        nc.vector.memset(zt[:, NCFG:], 1.0)

        acc = accp.tile([NCFG + 1, T], f32)

        for it, m in enumerate(sizes):
            c0 = it * 128
            tt = topop.tile([128, T], f32)
            tdma = nc.sync.dma_start(tt[:m], topo_d[c0:c0 + m, :])

            for g in range(3):
                qt = qpool.tile([128, 32 * QW], f32)   # 896 fp32 = 2 banks
                for h0, h1 in ((0, 512), (512, 896)):
                    nc.tensor.matmul(qt[:m, h0:h1],
                                     lhsT=ft[:, c0:c0 + m],
                                     rhs=mmat[:, g * 896 + h0:g * 896 + h1],
                                     start=True, stop=True)
                qv = qt[:m].rearrange("p (c w) -> p c w", w=QW)
                srb = srp.tile([128, 32, 14], f32)
                nc.scalar.copy(srb[:m], qv[:, :, 14:28])
                nc.vector.tensor_mul(pt[:m, g * 32:(g + 1) * 32, :],
                                     qv[:, :, 0:14], srb[:m])

            pv = pt[:m]
            nc.vector.tensor_sub(
                ns2[:m, 0:NCFG * 4].rearrange("p (c w) -> p c w", w=4),
                pv[:, :, 0:4], pv[:, :, 10:14])
            nc.vector.tensor_sub(num[:m, :, 0:3], pv[:, :, 4:7], pv[:, :, 7:10])
            nc.vector.tensor_mul(den[:m], ns2[:m, 0:NCFG * 4],
                                 ns2[:m, 1:NCFG * 4 + 1])
            nc.vector.tensor_scalar_max(den[:m], den[:m], DEN_CLAMP)
            # rr = den**-0.5 via exp(-0.5*ln(den)) -- both on ScalarE (one
            # table set), keeping the bottleneck DVE free
            nc.scalar.activation(lnd[:m], den[:m],
                                 mybir.ActivationFunctionType.Ln)
            nc.scalar.activation(rr[:m], lnd[:m],
                                 mybir.ActivationFunctionType.Exp, scale=-0.5)
            nc.vector.tensor_mul(cosw[:m], num[:m],
                                 rr[:m].rearrange("p (c w) -> p c w", w=4))
            if dbg and it == 0:
                nc.sync.dma_start(dbg_d[:m],
                                  cosw[:m].rearrange("p c w -> p (c w)"))
                for g in range(3):
                    nc.sync.dma_start(
                        dbgq_d[:m, g * 896:(g + 1) * 896].rearrange(
                            "p (c w) -> p c w", w=QW)[:, :, 0:14],
                        pt[:m, g * 32:(g + 1) * 32, :])
            nc.vector.tensor_reduce(zt[:m, 0:NCFG], cosw[:m],
                                    axis=mybir.AxisListType.X,
                                    op=mybir.AluOpType.add)
            nc.tensor.matmul(acc[:], lhsT=zt[:m], rhs=tt[:m],
                             start=(it == 0), stop=(it == ntiles - 1))

        accs = work.tile([NCFG + 1, T], f32)
        nc.vector.tensor_copy(accs[:], acc[:])
        nc.sync.dma_start(out_d[:], accs[:])

    # hardware allows at most one semaphore wait per instruction (two on
    # EventSemaphore); these Bacc passes legalize the Tile-emitted waits
    bass_rust.move_matmul_waits_to_ldweights(nc.m)
    bass_rust.generate_event_semaphores(nc)
    return nc


def _get_nc():
    if "nc" not in _CACHE:
        _CACHE["nc"] = _build_bass()
    return _CACHE["nc"]


def _host_reduce(acc_sum):
    """acc_sum [97, 256] float64 -> loss"""
    term1 = float((acc_sum[NCFG] * W1).sum())
    term2 = float(acc_sum[np.arange(NCFG), TOPO2TRI].sum())
    return term1 - term2


def _shard_inputs(off, topo, c):
    """Host-side marshalling for core c: slab offset rows, replicated
    pair-operand rows, topo shard."""
    o = np.empty((12, CELLS), dtype=np.float32)
    for e, (dx, dy, dz, ax) in enumerate(EDGES):
        o[e] = off[ax, WS * c + dx:WS * c + dx + WS,
                   dy:dy + H, dz:dz + D].reshape(CELLS)
    ia = np.array([a for a, b in PAIRS])
    ib = np.array([b for a, b in PAIRS])
    ones = np.ones((1, CELLS), np.float32)
    lab_l = np.concatenate([o[ia], ones, o], 0)          # [91, CELLS]
    lab_r = np.concatenate([o[ib], np.ones((13, CELLS), np.float32)], 0)
    return {
        "lab": np.ascontiguousarray(np.concatenate([lab_l, lab_r], axis=1)),
        "topo": np.ascontiguousarray(topo[CELLS * c:CELLS * (c + 1), :]),
        "mmat": MMAT_DEV,
    }


def kernel(off, topo):
    from concourse.bass_utils import run_bass_kernel_spmd

    off = np.ascontiguousarray(np.asarray(off), dtype=np.float32)
    topo = np.ascontiguousarray(np.asarray(topo), dtype=np.float32)
    assert off.shape == (3, W + 1, H + 1, D + 1)
    assert topo.shape == (N, T)

    nc = _get_nc()
    in_maps = [_shard_inputs(off, topo, c) for c in range(NCORES)]
    res = run_bass_kernel_spmd(nc, in_maps, core_ids=list(range(NCORES)))
    acc_sum = np.zeros((NCFG + 1, T), dtype=np.float64)
    for r in res.results:
        acc_sum += np.asarray(r["out"], dtype=np.float64)
    return np.float32(_host_reduce(acc_sum))



# revision 16
# speedup vs baseline: 8.2843x; 8.2843x over previous
"""Trainium2 Bass kernel for the CurvatureConstraint (marching-cubes curvature
loss) problem. Self-contained: rebuilds the deterministic topology tables,
compiles an 8-core SPMD Bass/Tile kernel, shards cells over the W axis, and
host-reduces the per-core partial accumulators to the scalar loss.

Formulation (validated on host to ~5e-5 vs the fp32 reference):
  Per cell, triangle (cfg,t): d1 = v(e1)-v(e0), d2 = v(e2)-v(e0) are LINEAR
  in the 12 edge offsets o, so the normal n = d1 x d2 is QUADRATIC in o.
  One PE matmul  F[cells,91] @ M[91, 588]  emits all normal components
  directly (F = [o_a*o_b pair products (78), 1, o (12)]), with fp16 inputs
  and fp32 PSUM accumulation.  Then per consecutive-triangle pair:
      ns2_t = |n_t|^2   (sum of squares -> never negative, no cancellation)
      rs_t  = rsqrt(ns2_t + 1e-4)          (ScalarE activation)
      cos_p = <n_t,n_u> * rs_t * rs_u      (|cos|<=1 by Cauchy-Schwarz, so
                                            rounding errors stay bounded)
      z[cfg] = sum_p cos_p
  Only the 69 cfgs with NTRI>=2 contribute (NTRI==1 has no pairs); cfgs are
  bucketed by NTRI (19 x k=4, 20 x k=3, 30 x k=2) and padded to a uniform
  [69,4,3] layout with zero M columns, which keeps every DVE op a single
  large affine access pattern.  PE: acc[70,256] += [z|1]^T @ topo, in fp8
  (statistically safe: topo enters linearly and errors average out).
  Host: loss = sum(W1 * acc[69]) - sum_cfg acc[cfg, g_cfg] over cores.
"""
import os
import sys
import numpy as np
import ml_dtypes

for _p in ("/opt/trn_rl_repo",):
    if _p not in sys.path and os.path.isdir(_p):
        sys.path.append(_p)

# ----------------------------------------------------------------------------
# Problem constants and deterministic tables (match reference.py exactly)
# ----------------------------------------------------------------------------
W = H = D = 40
T = 256
NCFG = 96
MAXT = 4
N = W * H * D

_rs = np.random.RandomState(0)
TOPO2TRI = _rs.randint(0, T, size=NCFG)
TRI_EDGES = _rs.rand(NCFG, MAXT, 12).argsort(-1)[..., :3]
_NTRI = _rs.randint(1, MAXT + 1, size=NCFG)

EDGES = [(0,0,0,0),(0,1,0,0),(0,0,1,0),(0,1,1,0),
         (0,0,0,1),(1,0,0,1),(0,0,1,1),(1,0,1,1),
         (0,0,0,2),(1,0,0,2),(0,1,0,2),(1,1,0,2)]
CORNER = np.array([[dx, dy, dz] for dx, dy, dz, ax in EDGES], dtype=np.float64)
AXIS_OF = np.array([ax for dx, dy, dz, ax in EDGES], dtype=np.int64)
AXES = np.eye(3)

NCORES = 8
WS = W // NCORES            # 5 planes of cells per core
CELLS = WS * H * D          # 8000
EPS = 1e-4                  # ns2 floor; also bounds rs<=100 (fp16-safe)

# Active cfgs bucketed by triangle count k (NTRI==1 contributes nothing)
ACT_CFGS = [c for k in (4, 3, 2) for c in range(NCFG) if _NTRI[c] == k]
KOF = [int(_NTRI[c]) for c in ACT_CFGS]
NACT = len(ACT_CFGS)                       # 69
NB4 = sum(1 for k in KOF if k == 4)        # 19
NB3 = sum(1 for k in KOF if k == 3)        # 20
NB2 = sum(1 for k in KOF if k == 2)        # 30
NQ = 3 * sum(KOF)                          # 588 matmul columns (compact)

# ---------------- feature basis: [o_a*o_b (78), 1, o (12)] ----------------
PAIRS = sorted({(min(a, b), max(a, b)) for a in range(12) for b in range(12)})
NPAIRF = len(PAIRS)         # 78
NF = NPAIRF + 1 + 12        # 91
PAIR_IDX = {p: i for i, p in enumerate(PAIRS)}
ONE_ROW = NPAIRF
O_ROW0 = NPAIRF + 1
IA = np.array([a for a, b in PAIRS])
IB = np.array([b for a, b in PAIRS])
# ia is 12 broadcast runs (a fixed), ib contiguous ranges a..11 within a run
IA_RUNS = []
_s = 0
for _i in range(1, NPAIRF + 1):
    if _i == NPAIRF or IA[_i] != IA[_s]:
        IA_RUNS.append((int(IA[_s]), _s, _i))
        _s = _i


def _lin_form(e0, e1):
    """d = v[e1]-v[e0]: (const c[3], {edge: coef[3]})."""
    c = CORNER[e1] - CORNER[e0]
    coeffs = {}
    coeffs[e1] = coeffs.get(e1, np.zeros(3)) + AXES[AXIS_OF[e1]]
    coeffs[e0] = coeffs.get(e0, np.zeros(3)) - AXES[AXIS_OF[e0]]
    return c, coeffs


def _comp_product(fA, a, fB, b):
    """Expansion of d1[a]*d2[b] (scalar component product) in the 91-basis."""
    cA, mA = fA
    cB, mB = fB
    v = np.zeros(NF)
    v[ONE_ROW] = cA[a] * cB[b]
    for e, ca in mA.items():
        v[O_ROW0 + e] += ca[a] * cB[b]
    for e, cb in mB.items():
        v[O_ROW0 + e] += cA[a] * cb[b]
    for ea, ca in mA.items():
        for eb, cb in mB.items():
            v[PAIR_IDX[(min(ea, eb), max(ea, eb))]] += ca[a] * cb[b]
    return v


def _build_mmat():
    """M [91, 588]: column (cfg,t,comp) emits normal component
    n_comp = d1[a]*d2[b] - d1[b]*d2[a], bucket-major compact layout."""
    cols = []
    for ci, cfg in enumerate(ACT_CFGS):
        for t in range(KOF[ci]):
            e0, e1, e2 = TRI_EDGES[cfg, t]
            d1 = _lin_form(e0, e1)
            d2 = _lin_form(e0, e2)
            for a, b in ((1, 2), (2, 0), (0, 1)):
                cols.append(_comp_product(d1, a, d2, b)
                            - _comp_product(d1, b, d2, a))
    return np.stack(cols, axis=1).astype(np.float32)

MMAT = _build_mmat()

# Only the topo columns referenced by active cfgs matter (59 of 256):
# ship/reduce just those.  term1 = sum_u W1U[u] * acc[NACT, u].
ACT_G = TOPO2TRI[ACT_CFGS]
USED_COLS = np.unique(ACT_G)
NU = len(USED_COLS)                        # 59
UIDX = np.searchsorted(USED_COLS, ACT_G)   # cfg -> used-col index
W1U = np.zeros(NU, dtype=np.float64)
for _ci in range(NACT):
    W1U[UIDX[_ci]] += KOF[_ci] - 1
NTILES = (CELLS + 127) // 128              # 63
CELLS_PAD = NTILES * 128                   # 8064 (device topo rows, padded)

# ----------------------------------------------------------------------------
# Bass kernel
# ----------------------------------------------------------------------------
_CACHE = {}


def _build_bass():
    import concourse.bass as bass
    import concourse.tile as tile
    import bass_rust
    from concourse import mybir
    from contextlib import ExitStack

    f32 = mybir.dt.float32
    f16 = mybir.dt.float16
    f8 = mybir.dt.float8e4
    cells = CELLS
    ntiles = (cells + 127) // 128
    sizes = [128] * (cells // 128) + ([cells % 128] if cells % 128 else [])

    nc = bass.Bass()
    # row 0 = ones (DMA-fed: non-DMA SBUF APs must start at partition
    # 0/32/64/96, so ft's ones row cannot come from a memset), rows 1:13 = o
    o16_d = nc.dram_tensor("o16", [13, cells], f16, kind="ExternalInput")
    topo_d = nc.dram_tensor("topo", [CELLS_PAD, NU], f8, kind="ExternalInput")
    mmat_d = nc.dram_tensor("mmat", [NF, NQ], f16, kind="ExternalInput")
    out_d = nc.dram_tensor("out", [NACT + 1, NU], f32, kind="ExternalOutput")

    with ExitStack() as ctx, nc.allow_low_precision(
            reason="fp16 chain validated on host sim (rel err 5e-5)"):
        tc = ctx.enter_context(tile.TileContext(nc))
        const = ctx.enter_context(tc.tile_pool(name="const", bufs=1))
        work = ctx.enter_context(tc.tile_pool(name="work", bufs=1))
        qpool = ctx.enter_context(tc.tile_pool(name="qp", bufs=2, space="PSUM"))
        accp = ctx.enter_context(tc.tile_pool(name="accp", bufs=1, space="PSUM"))

        mmat = const.tile([NF, NQ], f16)
        nc.sync.dma_start(mmat[:], mmat_d[:])

        # whole fp8 topo shard resident in SBUF (3.5KB/partition) via ONE
        # transposed DMA: cell (t*128+p) -> topoAll[p, t, :]
        topoAll = const.tile([128, NTILES, NU], f8)
        nc.sync.dma_start(
            topoAll[:],
            topo_d[:].rearrange("(t p) e -> p t e", p=128))

        # pair-product operands: ga = o[ia] (12 broadcast DMAs),
        # gb = o[ib] (12 contiguous-range DMAs)
        ga = const.tile([NPAIRF, cells], f16)
        gb = const.tile([NPAIRF, cells], f16)
        for a, s, e in IA_RUNS:
            src = o16_d[1 + a:2 + a, :].partition_broadcast(e - s)
            nc.sync.dma_start(ga[s:e, :], src)
            nc.sync.dma_start(gb[s:e, :], o16_d[1 + a:13, :])

        ft = const.tile([NF, cells], f16)
        nc.sync.dma_start(ft[ONE_ROW:NF, :], o16_d[:])
        # one DVE product builds all 78 pair-feature rows; the mmat "touch"
        # transitively orders the mmat DMA before the first matmul (codegen
        # allows at most one semaphore wait per instruction)
        scratch = work.tile([1, 4], f16)
        touch = nc.vector.tensor_copy(scratch[0:1, 0:1], mmat[0:1, 0:1])
        prod = nc.vector.tensor_mul(ft[0:NPAIRF, :], ga[:], gb[:])
        bass_rust.add_dep_helper(prod.ins, touch.ins, sync=False,
                                 reason="order mmat-touch before ft product")

        # persistent per-tile intermediates (DVE-serial, single buffers)
        qc = work.tile([128, NACT, 4, 3], f16)   # padded normals
        sqp = work.tile([128, NACT, 4, 4], f16)  # [..,3] slot holds EPS
        ns2 = work.tile([128, NACT, 4], f16)
        lnd = work.tile([128, NACT, 4], f16)
        rsq = work.tile([128, NACT, 4], f16)
        npr = work.tile([128, NACT, 3, 3], f16)
        nump = work.tile([128, NACT, 3], f16)
        rrp = work.tile([128, NACT, 3], f16)
        cosp = work.tile([128, NACT, 3], f16)
        ztf = work.tile([128, NACT], f16)
        zt = work.tile([128, NACT + 1], f8)
        nc.vector.memset(qc[:], 0.0)             # pad slots stay zero
        nc.vector.memset(sqp[:], EPS)            # [..,3] slots: ns2 += eps
        nc.vector.memset(zt[:, NACT:], 1.0)

        acc = accp.tile([NACT + 1, NU], f32)

        for it, m in enumerate(sizes):
            c0 = it * 128
            qt = qpool.tile([128, 1024], f32)    # 2 PSUM banks
            nc.tensor.matmul(qt[:m, 0:512], lhsT=ft[:, c0:c0 + m],
                             rhs=mmat[:, 0:512], start=True, stop=True)
            nc.tensor.matmul(qt[:m, 512:NQ], lhsT=ft[:, c0:c0 + m],
                             rhs=mmat[:, 512:NQ], start=True, stop=True)

            # scatter compact matmul columns into the padded [69,4,3] layout
            q4 = qt[:m, 0:12 * NB4]
            q3 = qt[:m, 12 * NB4:12 * NB4 + 9 * NB3].rearrange(
                "p (c w) -> p c w", w=9)
            q2 = qt[:m, 12 * NB4 + 9 * NB3:NQ].rearrange(
                "p (c w) -> p c w", w=6)
            nc.vector.tensor_copy(
                qc[:m, 0:NB4].rearrange("p c t x -> p (c t x)"), q4)
            nc.scalar.copy(
                qc[:m, NB4:NB4 + NB3, 0:3, :].rearrange(
                    "p c t x -> p c (t x)"), q3)
            nc.scalar.copy(
                qc[:m, NB4 + NB3:NACT, 0:2, :].rearrange(
                    "p c t x -> p c (t x)"), q2)

            nc.vector.tensor_mul(sqp[:m, :, :, 0:3], qc[:m], qc[:m])
            nc.vector.tensor_reduce(
                ns2[:m], sqp[:m].rearrange("p c t x -> p (c t) x"),
                axis=mybir.AxisListType.X, op=mybir.AluOpType.add)
            # rs = (ns2+eps)**-0.5 via exp(-0.5*ln(ns2+eps)) -- Rsqrt is
            # blocked (accuracy), Ln+Exp share one ScalarE table set; the
            # eps comes from the memset [..,3] slot summed by the reduce
            nc.scalar.activation(lnd[:m], ns2[:m],
                                 mybir.ActivationFunctionType.Ln)
            nc.scalar.activation(rsq[:m], lnd[:m],
                                 mybir.ActivationFunctionType.Exp, scale=-0.5)
            nc.vector.tensor_mul(npr[:m], qc[:m, :, 0:3, :], qc[:m, :, 1:4, :])
            nc.vector.tensor_reduce(
                nump[:m], npr[:m].rearrange("p c t x -> p (c t) x"),
                axis=mybir.AxisListType.X, op=mybir.AluOpType.add)
            nc.vector.tensor_mul(rrp[:m], rsq[:m, :, 0:3], rsq[:m, :, 1:4])
            nc.vector.tensor_mul(cosp[:m], nump[:m], rrp[:m])
            nc.vector.tensor_reduce(ztf[:m], cosp[:m],
                                    axis=mybir.AxisListType.X,
                                    op=mybir.AluOpType.add)
            nc.scalar.copy(zt[:m, 0:NACT], ztf[:m])
            nc.tensor.matmul(acc[:], lhsT=zt[:m], rhs=topoAll[:m, it, :],
                             start=(it == 0), stop=(it == ntiles - 1))

        accs = work.tile([NACT + 1, NU], f32)
        nc.vector.tensor_copy(accs[:], acc[:])
        nc.sync.dma_start(out_d[:], accs[:])

    # hardware allows at most one semaphore wait per instruction (two on
    # EventSemaphore); these Bacc passes legalize the Tile-emitted waits
    bass_rust.move_matmul_waits_to_ldweights(nc.m)
    bass_rust.generate_event_semaphores(nc)
    return nc


def _get_nc():
    if "nc" not in _CACHE:
        _CACHE["nc"] = _build_bass()
    return _CACHE["nc"]


def _host_reduce(acc_sum):
    """acc_sum [NACT+1, NU] float64 -> loss"""
    term1 = float((acc_sum[NACT] * W1U).sum())
    term2 = float(acc_sum[np.arange(NACT), UIDX].sum())
    return term1 - term2


def _marshal(off, topo):
    """Build the concatenated (8 cores stacked on axis 0) device inputs."""
    o16 = np.empty((NCORES * 13, CELLS), dtype=np.float16)
    for c in range(NCORES):
        o16[13 * c] = 1.0
        for e, (dx, dy, dz, ax) in enumerate(EDGES):
            o16[13 * c + 1 + e] = off[ax, WS * c + dx:WS * c + dx + WS,
                                      dy:dy + H, dz:dz + D].reshape(CELLS)
    tu8 = np.asarray(topo, dtype=np.float32).take(USED_COLS, axis=1).astype(
        ml_dtypes.float8_e4m3)
    topo8 = np.zeros((NCORES, CELLS_PAD, NU), dtype=ml_dtypes.float8_e4m3)
    topo8[:, 0:CELLS] = tu8.reshape(NCORES, CELLS, NU)
    return o16, topo8.reshape(NCORES * CELLS_PAD, NU)


def _get_runner():
    """Persistent jit of the SPMD bass executable (the same PJRT path
    run_bass_kernel_spmd takes under axon, minus the per-call re-trace),
    with the constant mmat operand left resident on the devices."""
    if "runner" in _CACHE:
        return _CACHE["runner"]

    import jax
    from jax.sharding import Mesh, PartitionSpec, NamedSharding
    from jax.experimental.shard_map import shard_map
    from concourse.bass2jax import (
        _bass_exec_p, install_neuronx_cc_hook, partition_id_tensor)
    from concourse import mybir

    nc = _get_nc()
    install_neuronx_cc_hook()
    pname = nc.partition_id_tensor.name if nc.partition_id_tensor else None
    in_names, out_names, out_avals, zero_outs = [], [], [], []
    for alloc in nc.m.functions[0].allocations:
        if not isinstance(alloc, mybir.MemoryLocationSet):
            continue
        name = alloc.memorylocations[0].name
        if alloc.kind == "ExternalInput":
            if name != pname:
                in_names.append(name)
        elif alloc.kind == "ExternalOutput":
            shape = tuple(alloc.tensor_shape)
            out_names.append(name)
            out_avals.append(jax.core.ShapedArray(shape, mybir.dt.np(alloc.dtype)))
            zero_outs.append(np.zeros(shape, mybir.dt.np(alloc.dtype)))
    n_params = len(in_names)
    n_outs = len(out_avals)
    all_names = in_names + out_names + ([pname] if pname else [])
    donate = tuple(range(n_params, n_params + n_outs))

    def _body(*args):
        operands = list(args)
        if pname is not None:
            operands.append(partition_id_tensor())
        return tuple(_bass_exec_p.bind(
            *operands, out_avals=tuple(out_avals), in_names=tuple(all_names),
            out_names=tuple(out_names), lowering_input_output_aliases=(),
            sim_require_finite=True, sim_require_nnan=True, nc=nc))

    devices = jax.devices()[:NCORES]
    mesh = Mesh(np.asarray(devices), ("core",))
    spec = (PartitionSpec("core"),) * (n_params + n_outs)
    fn = jax.jit(
        shard_map(_body, mesh=mesh, in_specs=spec,
                  out_specs=(PartitionSpec("core"),) * n_outs,
                  check_rep=False),
        donate_argnums=donate, keep_unused=True)

    mmat_dev = jax.device_put(
        np.concatenate([MMAT.astype(np.float16)] * NCORES, axis=0),
        NamedSharding(mesh, PartitionSpec("core")))

    def run(o16, topo8):
        by_name = {"o16": o16, "topo": topo8, "mmat": mmat_dev}
        args = [by_name[n] for n in in_names]
        zeros = [np.zeros((NCORES * z.shape[0], *z.shape[1:]), z.dtype)
                 for z in zero_outs]
        outs = fn(*args, *zeros)
        oi = out_names.index("out")
        return np.asarray(outs[oi]).reshape(NCORES, NACT + 1, NU)

    _CACHE["runner"] = run
    return run


def kernel(off, topo):
    off = np.ascontiguousarray(np.asarray(off), dtype=np.float32)
    assert off.shape == (3, W + 1, H + 1, D + 1)
    assert np.asarray(topo).shape == (N, T)

    run = _get_runner()
    o16, topo8 = _marshal(off, topo)
    accs = run(o16, topo8)
    acc_sum = accs.astype(np.float64).sum(axis=0)
    return np.float32(_host_reduce(acc_sum))
